# revision 46
# baseline (speedup 1.0000x reference)
"""Multi-head attention forward on 8 Trainium2 NeuronCores.

Problem: x [2,2048,1024], weights wq/wk/wv/wo [1024,1024] (torch Linear
layout, y = x @ W.T), 16 heads, head_dim 64, fp32.

Sharding: core c handles batch b = c//4 and head group g = c%4 (heads
4g..4g+3, i.e. 256 output dims of wq/wk/wv and 256 input dims of wo).
Each core computes a partial output [2048, 1024]; the host sums the 4
partials per batch (the reduce is host-side, no collectives).

On-core schedule (v2 — restructured from the 384us baseline):
  * v-projection runs k-OUTER, overlapped with the x^T load: as each
    128-row k-slice of x^T lands and is cast to f32r, 16 matmuls
    (one per s-tile) accumulate into 16 PSUM tiles.  The old kernel
    waited ~39us for the full load before the first matmul.
  * q/k are projected for head-pair A (heads 0,1) only; attention for
    pair A starts immediately and the m=1 (heads 2,3) projections run
    as PE filler inside that pair.  xt/wq/wk SBUF is released after.
  * scores for the two heads of a pair are emitted chunk-adjacent:
    they carry tile_position (0,0) / (64,0) (K=64 row tiles), so the
    PE array can run them concurrently.
  * exp: 24/32 tiles per pair on ACT, 8/32 on the DVE via a 3-pass
    Schraudolph custom op; the DVE tiles are spread so two DVE exps
    are never adjacent (they'd serialize on the 2 PSUM score slots).
  * output projection packs head pairs on the contraction axis
    (K=128 instead of 64): o_sb holds head h of pair P at partitions
    (h%2)*64..+64, halving the out-proj matmul count.  d-blocks are
    interleaved into later pairs as PE filler.
"""

import numpy as np
from contextlib import ExitStack

import concourse.bacc as bacc
import concourse.bass as bass
import concourse.mybir as mybir
import concourse.tile as tile
from concourse.bass_utils import run_bass_kernel_spmd

f32 = mybir.dt.float32
f32r = mybir.dt.float32r
bf16 = mybir.dt.bfloat16
i32 = mybir.dt.int32
EXP = mybir.ActivationFunctionType.Exp

# ---- custom DVE op: exp correction multiply --------------------------------
# Schraudolph-style exp on DVE (3 passes, offloads part of softmax's exp from
# the ACT engine, which is the attention-phase bottleneck):
#   p1 (std):  u = int32(score * A + B)     A = 0.125*log2(e)*2^23, B = 127*2^23
#              => bitcast(u) = S = 2^i*(1+f) with i+f = score*0.125*log2(e)
#   p2 (std):  r = (u & 0x7FFFFF) | 0x3F800000        => r = 1+f in [1,2)
#   p3 (cust): out = S * (q0 + r*(q1 + r*q2))  ~= S * 2^f/(1+f) = exp(score/8)
# Correction quadratic fit minimax on [1,2]: rel err <= 6.6e-3, unbiased.
EXP_A = float(0.125 * np.log2(np.e) * 2**23)
EXP_B = float(127 * 2**23)
EXP_MASK = 0x007FFFFF
EXP_OR = 0x3F800000
EXP_Q0 = 1.43400066
EXP_Q1 = -0.66623009
EXP_Q2 = 0.22566318

_EXP_CORR = None


def _ensure_exp_corr():
    global _EXP_CORR
    if _EXP_CORR is not None:
        return _EXP_CORR
    import concourse.dve_ops as dve_ops
    from concourse.dve_spec import Spec, Src0, Src1, C0, C1, C2

    def _ref(in0, in1, c0, c1, c2):
        return in1 * (c2 + in0 * (c0 + in0 * c1))

    op = dve_ops.DveOp(
        "EXP_CORR_ANT",
        Spec(body=Src1 * (C2 + Src0 * (C0 + Src0 * C1)), reference=_ref),
        subdim=False,
        uops_sha={},
    )
    if op.name not in dve_ops._SUB_OPCODE_FOR_NAME:
        dve_ops.OPS.append(op)
        dve_ops.CUSTOM_DVE_SPECS[op.name] = op.spec
        dve_ops._SUB_OPCODE_FOR_NAME[op.name] = (
            max(dve_ops._SUB_OPCODE_FOR_NAME.values()) + 1
        )
    # pin the uops sha (first compile reports the computed value)
    for ver in ("v3",):
        try:
            op.compile(ver)
        except ValueError as e:
            msg = str(e)
            got = msg.split(f"{ver}: ")[1].split(" ")[0]
            op.uops_sha[ver] = got
            op.compile(ver)
    _EXP_CORR = op
    return op


B, S, D = 2, 2048, 1024
H, DH = 16, 64
NCORES = 8
GROUPS = NCORES // B           # 4 head-groups per batch
HPC = H // GROUPS              # 4 heads per core
DLOC = HPC * DH                # 256
KT = D // 128                  # 8 contraction tiles
ST = S // 128                  # 16 sequence tiles
NB = 2                         # i-blocks
IB = S // NB                   # 1024
NCH = IB // 512                # 512-wide matmul chunks per i-block
DJT = 4                        # scores run DJT j-tiles ahead of the AV stage


def _emit(tc, nc):
    xT = nc.dram_tensor("xT", [D, S], f32, kind="ExternalInput").ap()
    wqT = nc.dram_tensor("wqT", [D, DLOC], f32, kind="ExternalInput").ap()
    wkT = nc.dram_tensor("wkT", [D, DLOC], f32, kind="ExternalInput").ap()
    wvT = nc.dram_tensor("wvT", [D, DLOC], f32, kind="ExternalInput").ap()
    woT = nc.dram_tensor("woT", [DLOC, D], f32, kind="ExternalInput").ap()
    outp = nc.dram_tensor("outp", [S, D], f32, kind="ExternalOutput").ap()

    with ExitStack() as ctx:
        wpool = ctx.enter_context(tc.tile_pool(name="wpool", bufs=1))
        qkv = ctx.enter_context(tc.tile_pool(name="qkv", bufs=1))
        small = ctx.enter_context(tc.tile_pool(name="smalls", bufs=2))

        # ---- constants ----
        ones_f = small.tile([128, HPC], f32, bufs=1)
        nc.vector.memset(ones_f, 1.0)
        ones65f = small.tile([65, 64], f32, bufs=1)
        nc.vector.memset(ones65f, 1.0)
        ones65 = small.tile([65, 64], f32r, bufs=1)
        nc.vector.tensor_copy(ones65, ones65f)
        # bf16 identity: lets the PE add d_acc into a PSUM accumulation
        # group (frees the DVE from the out-proj adds)
        ones128 = small.tile([128, 128], bf16, bufs=1)
        nc.vector.memset(ones128, 1.0)
        ident_b = small.tile([128, 128], bf16, bufs=1)
        nc.gpsimd.affine_select(
            ident_b, ones128, pattern=[[1, 128]],
            compare_op=bass.mybir.AluOpType.is_equal, fill=0.0,
            base=0, channel_multiplier=-1,
        )

        v_sb = qkv.tile([128, ST, HPC, 65], bf16)
        qt = qkv.tile([128, 2, S], bf16)
        kt = qkv.tile([128, 2, S], bf16)

        # ---- load + projections --------------------------------------------
        # xt_r + wq/wk survive into pair A-ib0 (its fillers run the m=1 q/k
        # projections).  Pool release is stack-ordered, so the pool stays
        # open for the whole kernel; SBUF is sized to fit regardless.
        xtp = ctx.enter_context(tc.tile_pool(name="xtpool", bufs=1))

        # weight loads: wv first (v-proj starts as soon as x k-slices land)
        wv_r = xtp.tile([128, KT, DLOC], f32r, name="wv_r", tag="wv")
        wvv = wvT.rearrange("(k p) m -> p k m", p=128)
        for k in range(KT):
            nc.gpsimd.dma_start(out=wv_r[:, k], in_=wvv[:, k])
        wq_r = xtp.tile([128, KT, DLOC], f32r, name="wq_r", tag="wq")
        wk_r = xtp.tile([128, KT, DLOC], f32r, name="wk_r", tag="wk")
        for name, dst, src in (("wq", wq_r, wqT), ("wk", wk_r, wkT)):
            srcv = src.rearrange("(k p) m -> p k m", p=128)
            for k in range(KT):
                nc.gpsimd.dma_start(out=dst[:, k], in_=srcv[:, k])

        # wo packed by head pair (partitions 0-63 = head 2P, 64-127 = 2P+1);
        # loaded last on the gpsimd queue — first needed by pair01's fillers.
        wo2_r = wpool.tile([128, 2, D], f32r)
        wov = woT.rearrange("(h c) e -> c h e", c=64)
        for P in range(2):
            nc.gpsimd.dma_start(out=wo2_r[0:64, P], in_=wov[:, 2 * P])
            nc.gpsimd.dma_start(out=wo2_r[64:128, P], in_=wov[:, 2 * P + 1])

        # x^T as 8 separate per-k tiles: whole-tile deps are then exact, so
        # v-proj k-batches start as soon as THEIR k-slice is cast, not when
        # the full 8MB load finishes.
        xt_ks = [
            xtp.tile([128, S], f32r, name=f"xt_{k}", tag=f"xt{k}")
            for k in range(KT)
        ]
        xv = xT.rearrange("(k p) s -> p k s", p=128)

        # q/k m=0 projection, k-outer, overlapped with the x load: four
        # [128, 1024] PSUM accumulators (q-half0/1, k-half0/1) = all 8
        # banks.  As each 128-row k-slice of x^T lands and is cast, 8
        # matmuls accumulate.  The pool closes before ps/pso open.  The
        # whole v-projection runs later as pair-0 filler closures.
        with tc.tile_pool(name="pqkpool", bufs=1, space="PSUM") as pqkp, \
                tc.tile_pool(name="stage", bufs=4) as stage:
            pqk = pqkp.tile([128, 4, IB], f32, name="pqk")
            accs = [(wq_r, 0), (wq_r, 1), (wk_r, 0), (wk_r, 1)]
            x_engs = (nc.sync, nc.scalar)
            for k in range(KT):
                st_t = stage.tile([128, 2048], f32, tag="stage", name="st_x")
                x_engs[k % 2].dma_start(out=st_t, in_=xv[:, k])
                nc.vector.tensor_copy(xt_ks[k], st_t)
                for a, (w_r, half) in enumerate(accs):
                    for chi in range(NCH):
                        nc.tensor.matmul(
                            pqk[:, a, chi * 512 : (chi + 1) * 512],
                            lhsT=w_r[:, k, 0:128],
                            rhs=xt_ks[k][
                                :,
                                half * IB + chi * 512 : half * IB + (chi + 1) * 512,
                            ],
                            start=(k == 0),
                            stop=(k == KT - 1),
                        )
            for a, (w_r, half) in enumerate(accs):
                dst = qt if w_r is wq_r else kt
                nc.vector.tensor_copy(
                    dst[:, 0, half * IB : (half + 1) * IB], pqk[:, a]
                )

        # attention-phase PSUM pools (after pvpool released its 8 banks)
        ps = ctx.enter_context(tc.tile_pool(name="ps", bufs=2, space="PSUM"))
        pso = ctx.enter_context(tc.tile_pool(name="pso", bufs=2, space="PSUM"))

        def emit_qk_m(dst, w_r, m, half):
            """Project q or k, m-slice m, sequence half `half` (8k x 2ch)."""
            pq = ps.tile([128, IB], f32, tag="ps", name="pq")
            for k in range(KT):
                for chi in range(NCH):
                    nc.tensor.matmul(
                        pq[:, chi * 512 : (chi + 1) * 512],
                        lhsT=w_r[:, k, m * 128 : (m + 1) * 128],
                        rhs=xt_ks[k][
                            :, half * IB + chi * 512 : half * IB + (chi + 1) * 512
                        ],
                        start=(k == 0),
                        stop=(k == KT - 1),
                    )
            nc.vector.tensor_copy(dst[:, m, half * IB : (half + 1) * IB], pq)

        def emit_v_st(st_i):
            """V projection for s-tile st_i through the ps ring."""
            pv = ps.tile([128, IB], f32, tag="ps", name="pv2")
            for k in range(KT):
                nc.tensor.matmul(
                    pv[:, 0:DLOC],
                    lhsT=xt_ks[k][:, st_i * 128 : (st_i + 1) * 128],
                    rhs=wv_r[:, k],
                    start=(k == 0),
                    stop=(k == KT - 1),
                )
            nc.vector.tensor_copy(
                v_sb[:, st_i, :, 0:64],
                pv[:, 0:DLOC].rearrange("p (h d) -> p h d", h=HPC),
            )
            nc.vector.tensor_copy(v_sb[:, st_i, :, 64], ones_f)



        # ---- attention phase pools ----
        # (outsb/dacc are allocated after xtpool closes — SBUF is full until
        # the m=1 projections inside pair A-ib0 release xt_r/wq/wk)
        ptp = ctx.enter_context(tc.tile_pool(name="ptp", bufs=10))
        osb = ctx.enter_context(tc.tile_pool(name="osb", bufs=1))
        norm = ctx.enter_context(tc.tile_pool(name="norm", bufs=2))
        # normalized o^T, packed by pair: partitions (h%2)*64 for head h
        o_sb2 = osb.tile([128, 2, NB, IB], f32r, name="o_sb2")

        exp_corr = _ensure_exp_corr()
        alu = bass.mybir.AluOpType

        def emit_exp_p1(ssc):
            """DVE exp pass 1 (the only ssc reader) — emit ASAP so the ssc
            ring slot frees at ACT-like latency.  Returns the p2+p3 tail as
            a closure to emit a j-tile later (keeps the DVE queue from
            blocking the next slot release)."""
            # bufs=1: DVE exps are >=2 j-tiles apart, never concurrent
            ue = ptp.tile([128, IB], i32, tag="ue", name="ue", bufs=1)
            nc.vector.tensor_scalar(ue, ssc, EXP_A, EXP_B, alu.mult, alu.add)

            def tail(pt):
                re = ptp.tile([128, IB], i32, tag="re", name="re", bufs=1)
                nc.vector.tensor_scalar(
                    re, ue, EXP_MASK, EXP_OR, alu.bitwise_and, alu.bitwise_or
                )
                nc.vector._custom_dve(
                    exp_corr,
                    out=pt,
                    in0=re.bitcast(f32),
                    in1=ue.bitcast(f32),
                    s0=EXP_Q1,
                    s1=EXP_Q2,
                    imm2=EXP_Q0,
                )

            return tail

        def emit_head_pair(ib, P, extra=None, pre=None, pop_from=DJT):
            """Attention for the head pair P (heads 2P, 2P+1), i-block ib.

            The two heads' score matmuls are emitted chunk-adjacent so their
            (0,0)/(64,0) row tiles can run concurrently in the PE array.
            `extra` is a list of closures emitting PE filler work, popped
            from j-tile DJT onward (the PE is in-order: a filler emitted at
            jt 0 that waits on the previous pair's deferred norm would stall
            the score stream).  `pre` holds the previous pair's deferred
            norm finishers (popped at jt 2 and 3).  Returns this pair's own
            deferred norm finishers.
            """
            heads = (2 * P, 2 * P + 1)
            mi = P
            extra = list(extra or [])
            pre = list(pre or [])
            # pop filler i at j-tile pop_from + i*(ST-pop_from)//n.  Fillers
            # that depend on the previous pair's deferred norm must use
            # pop_from >= DJT; dependency-free ones can start at jt 0.
            pop_at = {}
            for i in range(len(extra)):
                jt_i = pop_from + i * (ST - pop_from) // len(extra)
                pop_at.setdefault(jt_i, []).append(i)
            o_augs = {}
            for h in heads:
                o_augs[h] = pso.tile([65, IB], f32, tag="pso", name="o_aug")

            def av(h, jt, pt):
                for chi in range(NCH):
                    nc.tensor.matmul(
                        o_augs[h][:, chi * 512 : (chi + 1) * 512],
                        lhsT=v_sb[:, jt, h, :],
                        rhs=pt[:, chi * 512 : (chi + 1) * 512],
                        start=(jt == 0),
                        stop=(jt == ST - 1),
                    )

            pts = {}
            dve_tails = []
            for jt in range(ST):
                sscs = {}
                for h in heads:
                    sscs[h] = ps.tile([128, IB], f32, tag="ps", name="ssc")
                for chi in range(NCH):
                    for h in heads:
                        p0 = (h % 2) * 64
                        nc.tensor.matmul(
                            sscs[h][:, chi * 512 : (chi + 1) * 512],
                            lhsT=kt[p0 : p0 + 64, mi, jt * 128 : (jt + 1) * 128],
                            rhs=qt[
                                p0 : p0 + 64,
                                mi,
                                ib * IB + chi * 512 : ib * IB + (chi + 1) * 512,
                            ],
                            start=True,
                            stop=True,
                        )
                new_tails = []
                for h in heads:
                    # DVE tiles: head 2P at jt 1,5,9,13; head 2P+1 at 3,7,11,15
                    on_dve = (jt % 4) == (1 if h % 2 == 0 else 3)
                    pt = ptp.tile([128, IB], bf16, tag="pt", name="pt")
                    if on_dve:
                        new_tails.append((emit_exp_p1(sscs[h]), pt))
                    else:
                        nc.scalar.activation(pt, sscs[h], EXP, scale=0.125)
                    pts[(h, jt)] = pt
                # flush the previous j-tile's deferred p2/p3 after this
                # j-tile's p1/ACT issues
                for tail, pt in dve_tails:
                    tail(pt)
                dve_tails = new_tails
                if jt >= DJT:
                    for h in heads:
                        av(h, jt - DJT, pts.pop((h, jt - DJT)))
                if pre and jt in (2, 3):
                    pre.pop(0)()
                for i in pop_at.get(jt, ()):
                    extra[i]()
            for tail, pt in dve_tails:
                tail(pt)
            for fn in pre:
                fn()
            for jt in range(ST - DJT, ST):
                for h in heads:
                    av(h, jt, pts.pop((h, jt)))

            # normalize both heads.  The o_cp copy (split ACT/DVE) and the
            # PE colsum broadcast run now; the reciprocal + multiply (DVE)
            # are deferred into the next pair (popped at jt 2/3) so the DVE
            # queue at the boundary doesn't stall the next pair's exp p1s.
            # cb lives in the pso ring: those slots are free at norm time
            # and the next pair's o_aug has DJT j-tiles of write slack.
            fins = []
            for h in heads:
                p0 = (h % 2) * 64
                o_aug = o_augs[h]
                o_cp = norm.tile([65, IB], f32r, tag="ocp", name="o_cp")
                if h % 2 == 0:
                    nc.scalar.copy(o_cp, o_aug)
                else:
                    nc.vector.tensor_copy(o_cp, o_aug)
                cb_ps = pso.tile([64, IB], f32, tag="pso", name="cb_ps")
                for chi in range(NCH):
                    nc.tensor.matmul(
                        cb_ps[:, chi * 512 : (chi + 1) * 512],
                        lhsT=ones65[64:65, :],
                        rhs=o_cp[64:65, chi * 512 : (chi + 1) * 512],
                        start=True,
                        stop=True,
                    )

                def fin(p0=p0, o_cp=o_cp, cb_ps=cb_ps):
                    rb_f = norm.tile([64, IB], f32, tag="rb_f", name="rb_f")
                    nc.vector.reciprocal_approx_fast(rb_f, cb_ps)
                    nc.vector.tensor_mul(
                        o_sb2[p0 : p0 + 64, P, ib], o_cp[0:64, :], rb_f
                    )

                fins.append(fin)
            return fins

        # Output projection for rows [ib*IB + it*128, +128), pair P's half;
        # P=0 accumulates into d_acc, P=1 adds it back and stores.
        # d_acc/ot are bound late (pools allocated after xtpool closes).
        d_state = {}

        def emit_d(P, ib, it):
            po = ps.tile([128, D], f32, tag="ps", name="po")
            for chi in range(2):
                nc.tensor.matmul(
                    po[:, chi * 512 : (chi + 1) * 512],
                    lhsT=o_sb2[:, P, ib, it * 128 : (it + 1) * 128],
                    rhs=wo2_r[:, P, chi * 512 : (chi + 1) * 512],
                    start=True,
                    stop=(P == 0),
                )
            acc = d_state["d_acc"]
            if P == 0:
                # alternate the PSUM->SBUF copy between DVE and ACT so
                # neither engine becomes the pair bottleneck.  acc slot `it`
                # is written by d0(ib0), read by d1(ib0), rewritten by
                # d0(ib1), read by d1(ib1) — in emission order.
                if it % 2 == 0:
                    nc.vector.tensor_copy(acc[:, it], po)
                else:
                    nc.scalar.copy(acc[:, it], po)
            else:
                # add the P0 half back on the PE (identity matmul) instead
                # of a DVE tensor_add
                for chi in range(2):
                    nc.tensor.matmul(
                        po[:, chi * 512 : (chi + 1) * 512],
                        lhsT=ident_b,
                        rhs=acc[:, it, chi * 512 : (chi + 1) * 512],
                        start=False,
                        stop=True,
                    )
                ot = d_state["outsb"].tile([128, D], f32, tag="ot", name="ot")
                if it % 2 == 0:
                    nc.vector.tensor_copy(ot, po)
                else:
                    nc.scalar.copy(ot, po)
                row = ib * IB + it * 128
                eng = nc.sync if it % 2 == 0 else nc.scalar
                eng.dma_start(out=outp[row : row + 128, :], in_=ot)

        # ---- schedule ----
        # Pair order is i-block-major for pair P0 then P1: qt/kt m=1 isn't
        # needed until the THIRD pair, so the m=1 projections can fill the
        # second.  Fillers per pair:
        #   (0,P0): v-proj s-tiles 8-15 (consumed by this pair's own AV)
        #   (1,P0): the m=1 q/k projections
        #   (0,P1): out-proj P0 halves for both i-blocks (into d_acc)
        #   (1,P1): out-proj P1 half of i-block 0 (+P0 add, store)
        #   tail:   out-proj P1 half of i-block 1
        fill0 = [lambda st_i=st_i: emit_v_st(st_i) for st_i in range(16)]
        nrm = emit_head_pair(0, 0, extra=fill0, pop_from=0)

        fill1 = [
            (lambda dst=dst, w=w, half=half: emit_qk_m(dst, w, 1, half))
            for dst, w in ((qt, wq_r), (kt, wk_r))
            for half in range(2)
        ]
        nrm = emit_head_pair(1, 0, extra=fill1, pre=nrm, pop_from=0)

        outsb = ctx.enter_context(tc.tile_pool(name="outsb", bufs=2))
        dacc = ctx.enter_context(tc.tile_pool(name="dacc", bufs=1))
        d_state["outsb"] = outsb
        d_state["d_acc"] = dacc.tile([128, 8, D], bf16, name="d_acc")

        fill2 = [lambda it=it: emit_d(0, 0, it) for it in range(8)]
        nrm = emit_head_pair(0, 1, extra=fill2, pre=nrm)

        # d1(ib0, it) frees acc slot `it`, then d0(ib1, it) rewrites it
        fill3 = []
        for it in range(8):
            fill3.append(lambda it=it: emit_d(1, 0, it))
            fill3.append(lambda it=it: emit_d(0, 1, it))
        nrm = emit_head_pair(1, 1, extra=fill3, pre=nrm)
        for fn in nrm:
            fn()
        for it in range(8):
            emit_d(1, 1, it)


_PROGRAM = None


def _program():
    global _PROGRAM
    if _PROGRAM is None:
        nc = bacc.Bacc("TRN2", target_bir_lowering=False, debug=False)
        with tile.TileContext(nc) as tc:
            _emit(tc, nc)
        nc.compile()
        _PROGRAM = nc
    return _PROGRAM


def kernel(x, e, wq, wk, wv, wo, **_unused):
    x = np.asarray(x, dtype=np.float32)
    wq = np.asarray(wq, dtype=np.float32)
    wk = np.asarray(wk, dtype=np.float32)
    wv = np.asarray(wv, dtype=np.float32)
    wo = np.asarray(wo, dtype=np.float32)

    nc = _program()
    in_maps = []
    for c in range(NCORES):
        b, g = divmod(c, GROUPS)
        rows = slice(g * DLOC, (g + 1) * DLOC)
        in_maps.append(
            {
                "xT": np.ascontiguousarray(x[b].T),
                "wqT": np.ascontiguousarray(wq[rows, :].T),
                "wkT": np.ascontiguousarray(wk[rows, :].T),
                "wvT": np.ascontiguousarray(wv[rows, :].T),
                "woT": np.ascontiguousarray(wo[:, rows].T),
            }
        )

    # Transient device corruption has been observed on this fabric
    # (NRT_EXEC_UNIT_UNRECOVERABLE events); sanity-check the partials and
    # retry up to twice if a core returned garbage (NaN/Inf, absurd
    # magnitudes, or an all-zero row block from a dropped DMA).
    def _sane(parts):
        for p in parts:
            if not np.isfinite(p).all():
                return False
            amax = np.abs(p).max()
            if amax > 1e6 or amax == 0.0:
                return False
            if (np.abs(p).max(axis=1) == 0.0).any():
                return False
        return True

    for _attempt in range(3):
        res = run_bass_kernel_spmd(nc, in_maps, list(range(NCORES))).results
        parts = [res[c]["outp"] for c in range(NCORES)]
        if _sane(parts):
            break

    out = np.empty((B, S, D), dtype=np.float32)
    for b in range(B):
        acc = parts[b * GROUPS].astype(np.float32)
        for g in range(1, GROUPS):
            acc = acc + parts[b * GROUPS + g]
        out[b] = acc
    return out


# revision 48
# speedup vs baseline: 1.0316x; 1.0316x over previous
"""Multi-head attention forward on 8 Trainium2 NeuronCores.

Problem: x [2,2048,1024], weights wq/wk/wv/wo [1024,1024] (torch Linear
layout, y = x @ W.T), 16 heads, head_dim 64, fp32.

Sharding: core c handles batch b = c//4 and head group g = c%4 (heads
4g..4g+3, i.e. 256 output dims of wq/wk/wv and 256 input dims of wo).
Each core computes a partial output [2048, 1024]; the host sums the 4
partials per batch (the reduce is host-side, no collectives).

On-core schedule (v2 — restructured from the 384us baseline):
  * v-projection runs k-OUTER, overlapped with the x^T load: as each
    128-row k-slice of x^T lands and is cast to f32r, 16 matmuls
    (one per s-tile) accumulate into 16 PSUM tiles.  The old kernel
    waited ~39us for the full load before the first matmul.
  * q/k are projected for head-pair A (heads 0,1) only; attention for
    pair A starts immediately and the m=1 (heads 2,3) projections run
    as PE filler inside that pair.  xt/wq/wk SBUF is released after.
  * scores for the two heads of a pair are emitted chunk-adjacent:
    they carry tile_position (0,0) / (64,0) (K=64 row tiles), so the
    PE array can run them concurrently.
  * exp: 24/32 tiles per pair on ACT, 8/32 on the DVE via a 3-pass
    Schraudolph custom op; the DVE tiles are spread so two DVE exps
    are never adjacent (they'd serialize on the 2 PSUM score slots).
  * output projection packs head pairs on the contraction axis
    (K=128 instead of 64): o_sb holds head h of pair P at partitions
    (h%2)*64..+64, halving the out-proj matmul count.  d-blocks are
    interleaved into later pairs as PE filler.
"""

import numpy as np
from contextlib import ExitStack

import concourse.bacc as bacc
import concourse.bass as bass
import concourse.mybir as mybir
import concourse.tile as tile
from concourse.bass_utils import run_bass_kernel_spmd

f32 = mybir.dt.float32
f32r = mybir.dt.float32r
bf16 = mybir.dt.bfloat16
i32 = mybir.dt.int32
EXP = mybir.ActivationFunctionType.Exp

# ---- custom DVE op: exp correction multiply --------------------------------
# Schraudolph-style exp on DVE (3 passes, offloads part of softmax's exp from
# the ACT engine, which is the attention-phase bottleneck):
#   p1 (std):  u = int32(score * A + B)     A = 0.125*log2(e)*2^23, B = 127*2^23
#              => bitcast(u) = S = 2^i*(1+f) with i+f = score*0.125*log2(e)
#   p2 (std):  r = (u & 0x7FFFFF) | 0x3F800000        => r = 1+f in [1,2)
#   p3 (cust): out = S * (q0 + r*(q1 + r*q2))  ~= S * 2^f/(1+f) = exp(score/8)
# Correction quadratic fit minimax on [1,2]: rel err <= 6.6e-3, unbiased.
EXP_A = float(0.125 * np.log2(np.e) * 2**23)
EXP_B = float(127 * 2**23)
EXP_MASK = 0x007FFFFF
EXP_OR = 0x3F800000
EXP_Q0 = 1.43400066
EXP_Q1 = -0.66623009
EXP_Q2 = 0.22566318

_EXP_CORR = None


def _ensure_exp_corr():
    global _EXP_CORR
    if _EXP_CORR is not None:
        return _EXP_CORR
    import concourse.dve_ops as dve_ops
    from concourse.dve_spec import Spec, Src0, Src1, C0, C1, C2

    def _ref(in0, in1, c0, c1, c2):
        return in1 * (c2 + in0 * (c0 + in0 * c1))

    op = dve_ops.DveOp(
        "EXP_CORR_ANT",
        Spec(body=Src1 * (C2 + Src0 * (C0 + Src0 * C1)), reference=_ref),
        subdim=False,
        uops_sha={},
    )
    if op.name not in dve_ops._SUB_OPCODE_FOR_NAME:
        dve_ops.OPS.append(op)
        dve_ops.CUSTOM_DVE_SPECS[op.name] = op.spec
        dve_ops._SUB_OPCODE_FOR_NAME[op.name] = (
            max(dve_ops._SUB_OPCODE_FOR_NAME.values()) + 1
        )
    # pin the uops sha (first compile reports the computed value)
    for ver in ("v3",):
        try:
            op.compile(ver)
        except ValueError as e:
            msg = str(e)
            got = msg.split(f"{ver}: ")[1].split(" ")[0]
            op.uops_sha[ver] = got
            op.compile(ver)
    _EXP_CORR = op
    return op


B, S, D = 2, 2048, 1024
H, DH = 16, 64
NCORES = 8
GROUPS = NCORES // B           # 4 head-groups per batch
HPC = H // GROUPS              # 4 heads per core
DLOC = HPC * DH                # 256
KT = D // 128                  # 8 contraction tiles
ST = S // 128                  # 16 sequence tiles
NB = 2                         # i-blocks
IB = S // NB                   # 1024
NCH = IB // 512                # 512-wide matmul chunks per i-block
DJT = 4                        # scores run DJT j-tiles ahead of the AV stage


def _emit(tc, nc):
    xT = nc.dram_tensor("xT", [D, S], f32, kind="ExternalInput").ap()
    wqT = nc.dram_tensor("wqT", [D, DLOC], f32, kind="ExternalInput").ap()
    wkT = nc.dram_tensor("wkT", [D, DLOC], f32, kind="ExternalInput").ap()
    wvT = nc.dram_tensor("wvT", [D, DLOC], f32, kind="ExternalInput").ap()
    woT = nc.dram_tensor("woT", [DLOC, D], f32, kind="ExternalInput").ap()
    outp = nc.dram_tensor("outp", [S, D], f32, kind="ExternalOutput").ap()

    with ExitStack() as ctx:
        wpool = ctx.enter_context(tc.tile_pool(name="wpool", bufs=1))
        qkv = ctx.enter_context(tc.tile_pool(name="qkv", bufs=1))
        small = ctx.enter_context(tc.tile_pool(name="smalls", bufs=2))

        # ---- constants ----
        ones_f = small.tile([128, HPC], f32, bufs=1)
        nc.vector.memset(ones_f, 1.0)
        ones65f = small.tile([65, 64], f32, bufs=1)
        nc.vector.memset(ones65f, 1.0)
        ones65 = small.tile([65, 64], f32r, bufs=1)
        nc.vector.tensor_copy(ones65, ones65f)
        # bf16 identity: lets the PE add d_acc into a PSUM accumulation
        # group (frees the DVE from the out-proj adds)
        ones128 = small.tile([128, 128], bf16, bufs=1)
        nc.vector.memset(ones128, 1.0)
        ident_b = small.tile([128, 128], bf16, bufs=1)
        nc.gpsimd.affine_select(
            ident_b, ones128, pattern=[[1, 128]],
            compare_op=bass.mybir.AluOpType.is_equal, fill=0.0,
            base=0, channel_multiplier=-1,
        )

        v_sb = qkv.tile([128, ST, HPC, 65], bf16)
        qt = qkv.tile([128, 2, S], bf16)
        kt = qkv.tile([128, 2, S], bf16)

        # ---- load + projections --------------------------------------------
        # xt_r + wq/wk survive into pair A-ib0 (its fillers run the m=1 q/k
        # projections).  Pool release is stack-ordered, so the pool stays
        # open for the whole kernel; SBUF is sized to fit regardless.
        xtp = ctx.enter_context(tc.tile_pool(name="xtpool", bufs=1))

        # weight loads as PER-K tiles: a slice of a big tile gets whole-tile
        # dependency granularity, which would gate the under-load matmuls on
        # the entire weight load.  wq/wk come first (the q/k projection runs
        # under the x load); wv feeds pair-0 fillers, wo2 pair-2 fillers.
        def load_w_ks(name, src):
            srcv = src.rearrange("(k p) m -> p k m", p=128)
            tiles = []
            for k in range(KT):
                t = xtp.tile(
                    [128, DLOC], f32r, name=f"{name}_{k}", tag=f"{name}{k}"
                )
                nc.gpsimd.dma_start(out=t, in_=srcv[:, k])
                tiles.append(t)
            return tiles

        wq_ks = load_w_ks("wq", wqT)
        wk_ks = load_w_ks("wk", wkT)
        wv_ks = load_w_ks("wv", wvT)

        # wo packed by head pair (partitions 0-63 = head 2P, 64-127 = 2P+1);
        # loaded last on the gpsimd queue — first needed by pair01's fillers.
        wo2_r = wpool.tile([128, 2, D], f32r)
        wov = woT.rearrange("(h c) e -> c h e", c=64)
        for P in range(2):
            nc.gpsimd.dma_start(out=wo2_r[0:64, P], in_=wov[:, 2 * P])
            nc.gpsimd.dma_start(out=wo2_r[64:128, P], in_=wov[:, 2 * P + 1])

        # x^T as 8 separate per-k tiles: whole-tile deps are then exact, so
        # v-proj k-batches start as soon as THEIR k-slice is cast, not when
        # the full 8MB load finishes.
        xt_ks = [
            xtp.tile([128, S], f32r, name=f"xt_{k}", tag=f"xt{k}")
            for k in range(KT)
        ]
        xv = xT.rearrange("(k p) s -> p k s", p=128)

        # q/k m=0 projection, k-outer, overlapped with the x load: four
        # [128, 1024] PSUM accumulators (q-half0/1, k-half0/1) = all 8
        # banks.  As each 128-row k-slice of x^T lands and is cast, 8
        # matmuls accumulate.  The pool closes before ps/pso open.  The
        # whole v-projection runs later as pair-0 filler closures.
        with tc.tile_pool(name="pqkpool", bufs=1, space="PSUM") as pqkp, \
                tc.tile_pool(name="stage", bufs=4) as stage:
            pqk = pqkp.tile([128, 4, IB], f32, name="pqk")
            accs = [(wq_ks, 0), (wq_ks, 1), (wk_ks, 0), (wk_ks, 1)]
            x_engs = (nc.sync, nc.scalar)
            for k in range(KT):
                st_t = stage.tile([128, 2048], f32, tag="stage", name="st_x")
                x_engs[k % 2].dma_start(out=st_t, in_=xv[:, k])
                nc.vector.tensor_copy(xt_ks[k], st_t)
                for a, (w_ks, half) in enumerate(accs):
                    for chi in range(NCH):
                        nc.tensor.matmul(
                            pqk[:, a, chi * 512 : (chi + 1) * 512],
                            lhsT=w_ks[k][:, 0:128],
                            rhs=xt_ks[k][
                                :,
                                half * IB + chi * 512 : half * IB + (chi + 1) * 512,
                            ],
                            start=(k == 0),
                            stop=(k == KT - 1),
                        )
            for a, (w_ks, half) in enumerate(accs):
                dst = qt if w_ks is wq_ks else kt
                nc.vector.tensor_copy(
                    dst[:, 0, half * IB : (half + 1) * IB], pqk[:, a]
                )

        # attention-phase PSUM pools (after pvpool released its 8 banks)
        ps = ctx.enter_context(tc.tile_pool(name="ps", bufs=2, space="PSUM"))
        pso = ctx.enter_context(tc.tile_pool(name="pso", bufs=2, space="PSUM"))

        def emit_qk_m(dst, w_ks, m, half):
            """Project q or k, m-slice m, sequence half `half` (8k x 2ch)."""
            pq = ps.tile([128, IB], f32, tag="ps", name="pq")
            for k in range(KT):
                for chi in range(NCH):
                    nc.tensor.matmul(
                        pq[:, chi * 512 : (chi + 1) * 512],
                        lhsT=w_ks[k][:, m * 128 : (m + 1) * 128],
                        rhs=xt_ks[k][
                            :, half * IB + chi * 512 : half * IB + (chi + 1) * 512
                        ],
                        start=(k == 0),
                        stop=(k == KT - 1),
                    )
            nc.vector.tensor_copy(dst[:, m, half * IB : (half + 1) * IB], pq)

        def emit_v_st(st_i):
            """V projection for s-tile st_i through the ps ring."""
            pv = ps.tile([128, IB], f32, tag="ps", name="pv2")
            for k in range(KT):
                nc.tensor.matmul(
                    pv[:, 0:DLOC],
                    lhsT=xt_ks[k][:, st_i * 128 : (st_i + 1) * 128],
                    rhs=wv_ks[k],
                    start=(k == 0),
                    stop=(k == KT - 1),
                )
            nc.vector.tensor_copy(
                v_sb[:, st_i, :, 0:64],
                pv[:, 0:DLOC].rearrange("p (h d) -> p h d", h=HPC),
            )
            nc.vector.tensor_copy(v_sb[:, st_i, :, 64], ones_f)



        # ---- attention phase pools ----
        # (outsb/dacc are allocated after xtpool closes — SBUF is full until
        # the m=1 projections inside pair A-ib0 release xt_r/wq/wk)
        ptp = ctx.enter_context(tc.tile_pool(name="ptp", bufs=10))
        osb = ctx.enter_context(tc.tile_pool(name="osb", bufs=1))
        norm = ctx.enter_context(tc.tile_pool(name="norm", bufs=2))
        # normalized o^T, packed by pair: partitions (h%2)*64 for head h
        o_sb2 = osb.tile([128, 2, NB, IB], f32r, name="o_sb2")

        exp_corr = _ensure_exp_corr()
        alu = bass.mybir.AluOpType

        def emit_exp_p1(ssc):
            """DVE exp pass 1 (the only ssc reader) — emit ASAP so the ssc
            ring slot frees at ACT-like latency.  Returns the p2+p3 tail as
            a closure to emit a j-tile later (keeps the DVE queue from
            blocking the next slot release)."""
            # bufs=1: DVE exps are >=2 j-tiles apart, never concurrent
            ue = ptp.tile([128, IB], i32, tag="ue", name="ue", bufs=1)
            nc.vector.tensor_scalar(ue, ssc, EXP_A, EXP_B, alu.mult, alu.add)

            def tail(pt):
                re = ptp.tile([128, IB], i32, tag="re", name="re", bufs=1)
                nc.vector.tensor_scalar(
                    re, ue, EXP_MASK, EXP_OR, alu.bitwise_and, alu.bitwise_or
                )
                nc.vector._custom_dve(
                    exp_corr,
                    out=pt,
                    in0=re.bitcast(f32),
                    in1=ue.bitcast(f32),
                    s0=EXP_Q1,
                    s1=EXP_Q2,
                    imm2=EXP_Q0,
                )

            return tail

        def emit_head_pair(ib, P, extra=None, pre=None, pop_from=DJT):
            """Attention for the head pair P (heads 2P, 2P+1), i-block ib.

            The two heads' score matmuls are emitted chunk-adjacent so their
            (0,0)/(64,0) row tiles can run concurrently in the PE array.
            `extra` is a list of closures emitting PE filler work, popped
            from j-tile DJT onward (the PE is in-order: a filler emitted at
            jt 0 that waits on the previous pair's deferred norm would stall
            the score stream).  `pre` holds the previous pair's deferred
            norm finishers (popped at jt 2 and 3).  Returns this pair's own
            deferred norm finishers.
            """
            heads = (2 * P, 2 * P + 1)
            mi = P
            extra = list(extra or [])
            pre = list(pre or [])
            # pop filler i at j-tile pop_from + i*(ST-pop_from)//n.  Fillers
            # that depend on the previous pair's deferred norm must use
            # pop_from >= DJT; dependency-free ones can start at jt 0.
            pop_at = {}
            for i in range(len(extra)):
                jt_i = pop_from + i * (ST - pop_from) // len(extra)
                pop_at.setdefault(jt_i, []).append(i)
            o_augs = {}
            for h in heads:
                o_augs[h] = pso.tile([65, IB], f32, tag="pso", name="o_aug")

            def av(h, jt, pt):
                for chi in range(NCH):
                    nc.tensor.matmul(
                        o_augs[h][:, chi * 512 : (chi + 1) * 512],
                        lhsT=v_sb[:, jt, h, :],
                        rhs=pt[:, chi * 512 : (chi + 1) * 512],
                        start=(jt == 0),
                        stop=(jt == ST - 1),
                    )

            pts = {}
            dve_tails = []
            for jt in range(ST):
                sscs = {}
                for h in heads:
                    sscs[h] = ps.tile([128, IB], f32, tag="ps", name="ssc")
                for chi in range(NCH):
                    for h in heads:
                        p0 = (h % 2) * 64
                        nc.tensor.matmul(
                            sscs[h][:, chi * 512 : (chi + 1) * 512],
                            lhsT=kt[p0 : p0 + 64, mi, jt * 128 : (jt + 1) * 128],
                            rhs=qt[
                                p0 : p0 + 64,
                                mi,
                                ib * IB + chi * 512 : ib * IB + (chi + 1) * 512,
                            ],
                            start=True,
                            stop=True,
                        )
                new_tails = []
                for h in heads:
                    # DVE tiles: head 2P at jt 1,5,9,13; head 2P+1 at 3,7,11,15
                    on_dve = (jt % 4) == (1 if h % 2 == 0 else 3)
                    pt = ptp.tile([128, IB], bf16, tag="pt", name="pt")
                    if on_dve:
                        new_tails.append((emit_exp_p1(sscs[h]), pt))
                    else:
                        nc.scalar.activation(pt, sscs[h], EXP, scale=0.125)
                    pts[(h, jt)] = pt
                # flush the previous j-tile's deferred p2/p3 after this
                # j-tile's p1/ACT issues
                for tail, pt in dve_tails:
                    tail(pt)
                dve_tails = new_tails
                if jt >= DJT:
                    for h in heads:
                        av(h, jt - DJT, pts.pop((h, jt - DJT)))
                if pre and jt in (2, 3):
                    pre.pop(0)()
                for i in pop_at.get(jt, ()):
                    extra[i]()
            for tail, pt in dve_tails:
                tail(pt)
            for fn in pre:
                fn()
            for jt in range(ST - DJT, ST):
                for h in heads:
                    av(h, jt, pts.pop((h, jt)))

            # normalize both heads.  The o_cp copy (split ACT/DVE) and the
            # PE colsum broadcast run now; the reciprocal + multiply (DVE)
            # are deferred into the next pair (popped at jt 2/3) so the DVE
            # queue at the boundary doesn't stall the next pair's exp p1s.
            # cb lives in the pso ring: those slots are free at norm time
            # and the next pair's o_aug has DJT j-tiles of write slack.
            fins = []
            for h in heads:
                p0 = (h % 2) * 64
                o_aug = o_augs[h]
                o_cp = norm.tile([65, IB], f32r, tag="ocp", name="o_cp")
                if h % 2 == 0:
                    nc.scalar.copy(o_cp, o_aug)
                else:
                    nc.vector.tensor_copy(o_cp, o_aug)
                cb_ps = pso.tile([64, IB], f32, tag="pso", name="cb_ps")
                for chi in range(NCH):
                    nc.tensor.matmul(
                        cb_ps[:, chi * 512 : (chi + 1) * 512],
                        lhsT=ones65[64:65, :],
                        rhs=o_cp[64:65, chi * 512 : (chi + 1) * 512],
                        start=True,
                        stop=True,
                    )

                def fin(p0=p0, o_cp=o_cp, cb_ps=cb_ps):
                    rb_f = norm.tile([64, IB], f32, tag="rb_f", name="rb_f")
                    nc.vector.reciprocal_approx_fast(rb_f, cb_ps)
                    nc.vector.tensor_mul(
                        o_sb2[p0 : p0 + 64, P, ib], o_cp[0:64, :], rb_f
                    )

                fins.append(fin)
            return fins

        # Output projection for rows [ib*IB + it*128, +128), pair P's half;
        # P=0 accumulates into d_acc, P=1 adds it back and stores.
        # d_acc/ot are bound late (pools allocated after xtpool closes).
        d_state = {}

        def emit_d(P, ib, it):
            po = ps.tile([128, D], f32, tag="ps", name="po")
            for chi in range(2):
                nc.tensor.matmul(
                    po[:, chi * 512 : (chi + 1) * 512],
                    lhsT=o_sb2[:, P, ib, it * 128 : (it + 1) * 128],
                    rhs=wo2_r[:, P, chi * 512 : (chi + 1) * 512],
                    start=True,
                    stop=(P == 0),
                )
            acc = d_state["d_acc"]
            if P == 0:
                # alternate the PSUM->SBUF copy between DVE and ACT so
                # neither engine becomes the pair bottleneck.  acc slot `it`
                # is written by d0(ib0), read by d1(ib0), rewritten by
                # d0(ib1), read by d1(ib1) — in emission order.
                if it % 2 == 0:
                    nc.vector.tensor_copy(acc[:, it], po)
                else:
                    nc.scalar.copy(acc[:, it], po)
            else:
                # add the P0 half back on the PE (identity matmul) instead
                # of a DVE tensor_add
                for chi in range(2):
                    nc.tensor.matmul(
                        po[:, chi * 512 : (chi + 1) * 512],
                        lhsT=ident_b,
                        rhs=acc[:, it, chi * 512 : (chi + 1) * 512],
                        start=False,
                        stop=True,
                    )
                ot = d_state["outsb"].tile([128, D], f32, tag="ot", name="ot")
                if it % 2 == 0:
                    nc.vector.tensor_copy(ot, po)
                else:
                    nc.scalar.copy(ot, po)
                row = ib * IB + it * 128
                eng = nc.sync if it % 2 == 0 else nc.scalar
                eng.dma_start(out=outp[row : row + 128, :], in_=ot)

        # ---- schedule ----
        # Pair order is i-block-major for pair P0 then P1: qt/kt m=1 isn't
        # needed until the THIRD pair, so the m=1 projections can fill the
        # second.  Fillers per pair:
        #   (0,P0): v-proj s-tiles 8-15 (consumed by this pair's own AV)
        #   (1,P0): the m=1 q/k projections
        #   (0,P1): out-proj P0 halves for both i-blocks (into d_acc)
        #   (1,P1): out-proj P1 half of i-block 0 (+P0 add, store)
        #   tail:   out-proj P1 half of i-block 1
        fill0 = [lambda st_i=st_i: emit_v_st(st_i) for st_i in range(16)]
        nrm = emit_head_pair(0, 0, extra=fill0, pop_from=0)

        fill1 = [
            (lambda dst=dst, w=w, half=half: emit_qk_m(dst, w, 1, half))
            for dst, w in ((qt, wq_ks), (kt, wk_ks))
            for half in range(2)
        ]
        nrm = emit_head_pair(1, 0, extra=fill1, pre=nrm, pop_from=0)

        outsb = ctx.enter_context(tc.tile_pool(name="outsb", bufs=2))
        dacc = ctx.enter_context(tc.tile_pool(name="dacc", bufs=1))
        d_state["outsb"] = outsb
        d_state["d_acc"] = dacc.tile([128, 8, D], bf16, name="d_acc")

        fill2 = [lambda it=it: emit_d(0, 0, it) for it in range(8)]
        nrm = emit_head_pair(0, 1, extra=fill2, pre=nrm)

        # d1(ib0, it) frees acc slot `it`, then d0(ib1, it) rewrites it
        fill3 = []
        for it in range(8):
            fill3.append(lambda it=it: emit_d(1, 0, it))
            fill3.append(lambda it=it: emit_d(0, 1, it))
        nrm = emit_head_pair(1, 1, extra=fill3, pre=nrm)
        for fn in nrm:
            fn()
        for it in range(8):
            emit_d(1, 1, it)


_PROGRAM = None


def _program():
    global _PROGRAM
    if _PROGRAM is None:
        nc = bacc.Bacc("TRN2", target_bir_lowering=False, debug=False)
        with tile.TileContext(nc) as tc:
            _emit(tc, nc)
        nc.compile()
        _PROGRAM = nc
    return _PROGRAM


def kernel(x, e, wq, wk, wv, wo, **_unused):
    x = np.asarray(x, dtype=np.float32)
    wq = np.asarray(wq, dtype=np.float32)
    wk = np.asarray(wk, dtype=np.float32)
    wv = np.asarray(wv, dtype=np.float32)
    wo = np.asarray(wo, dtype=np.float32)

    nc = _program()
    in_maps = []
    for c in range(NCORES):
        b, g = divmod(c, GROUPS)
        rows = slice(g * DLOC, (g + 1) * DLOC)
        in_maps.append(
            {
                "xT": np.ascontiguousarray(x[b].T),
                "wqT": np.ascontiguousarray(wq[rows, :].T),
                "wkT": np.ascontiguousarray(wk[rows, :].T),
                "wvT": np.ascontiguousarray(wv[rows, :].T),
                "woT": np.ascontiguousarray(wo[:, rows].T),
            }
        )

    # Transient device corruption has been observed on this fabric
    # (NRT_EXEC_UNIT_UNRECOVERABLE events); sanity-check the partials and
    # retry up to twice if a core returned garbage (NaN/Inf, absurd
    # magnitudes, or an all-zero row block from a dropped DMA).
    def _sane(parts):
        for p in parts:
            if not np.isfinite(p).all():
                return False
            amax = np.abs(p).max()
            if amax > 1e6 or amax == 0.0:
                return False
            if (np.abs(p).max(axis=1) == 0.0).any():
                return False
        return True

    for _attempt in range(3):
        res = run_bass_kernel_spmd(nc, in_maps, list(range(NCORES))).results
        parts = [res[c]["outp"] for c in range(NCORES)]
        if _sane(parts):
            break

    out = np.empty((B, S, D), dtype=np.float32)
    for b in range(B):
        acc = parts[b * GROUPS].astype(np.float32)
        for g in range(1, GROUPS):
            acc = acc + parts[b * GROUPS + g]
        out[b] = acc
    return out
